# revision 13
# baseline (speedup 1.0000x reference)
"""Trainium2 Bass kernel for nn_CASCADES_v8_ResonantCore (moe_routing):

Computation (per batch b):
    centroid = 0.7*x[b,-1,:] + 0.3*mean_s(x[b])
    w = softmax(cos_sim(centroid, core_keys)/TEMP)      # [K]
    Lam = sum_k w[k] * core_pool[k]                     # [R,R]
    out[b] = ((x[b] @ V^T) @ Lam^T) @ U^T               # [S,D]

Strategy (8 cores, data-parallel over (batch, seq-half)):
  - Host: exact f64 routing; W_b = (U @ Lam_b)^T folded to one [R, D]
    weight per batch.  Output is written int8 with a per-column scale
    s_d = 8*sigma_d/127 (sigma_d^2 = W_d^T (V V^T) W_d) folded into
    the weight; host dequantizes.  f32->int8 on DVE/ACT rounds-to-
    nearest and saturates (HW-probed), so |err| <= s_d/2 ~ 0.03 sigma.
  - ALL DMA rides the single SP HWDGE ring, which executes FIFO:
    consts, then the whole 16.8 MiB x stream as 32x512 KiB chunks,
    then the int8 writes.  This gives reads strict priority (measured
    ~420 GB/s single-ring), so the tail mm2 never waits on a
    read/write-interleaved stream.
  - Device per seq-group of 512 rows: 32 accumulating [128,8]x[128,512]
    matmuls -> xv^T [8,512]; a one-matmul partition-replication with
    rp1 -> xvr [128,512]; 4x8 expansion matmuls; paired-bank drains
    ([128,1024] f32->int8 per copy, alternating DVE/ACT).  PE order is
    software-pipelined (mm1 of group g+1 before mm2 of group g).
  - HBM traffic per core: 16.8 MiB read + 8.4 MiB write ~= 25.3 MiB.
"""

import sys

sys.path.insert(0, "/opt/trn_rl_repo")

import contextlib

import ml_dtypes
import numpy as np

import concourse.bass as bass  # noqa: F401  (registers bass types)
import concourse.tile as tile
from concourse import bacc, mybir
from concourse.bass_utils import run_bass_kernel_spmd

BF16 = ml_dtypes.bfloat16

B, S, D, R, K = 4, 4096, 4096, 8, 4
NCORES = 8
SH = S // 2     # 2048 seq rows per core
G = 4           # seq groups per core
SG = SH // G    # 512 seq rows per group
NCH = D // 128  # 32 d-chunks
TPG = 2         # x tiles per group ([128, 8192] each)
CPT = NCH // TPG  # 16 d-chunks per x tile
NSUB = 4        # read sub-DMAs per tile (512 KiB each)
CPS = CPT // NSUB  # 4 d-chunks per sub-DMA
NSX = SG // 128   # 4 output strips per group
NDJ = D // 1024   # 4 paired-bank drain units per strip
EPS, TEMP = 1e-8, 0.05
QC = 8.0        # int8 scale: s_d = QC * sigma_d / 127

_cache = {}


def build_fused():
    """xtp [1024, 8192] bf16, vt [128, 256] bf16, wt [8, 4096] bf16
    (per-column-scaled W'), rp1/rp16 [8, 128] bf16 -> out [128, 65536]
    int8 with out[p, (g*4+q)*4096 + d] = out_rows[g*512 + q*128 + p, d]."""
    nc = bacc.Bacc("TRN2", target_bir_lowering=False, debug=False)
    xtp = nc.dram_tensor(
        "xtp", [G * TPG * 128, CPT * SG], mybir.dt.bfloat16, kind="ExternalInput"
    ).ap()
    vt = nc.dram_tensor("vt", [128, NCH * R], mybir.dt.bfloat16, kind="ExternalInput").ap()
    wt = nc.dram_tensor("wt", [R, D], mybir.dt.bfloat16, kind="ExternalInput").ap()
    rp1 = nc.dram_tensor("rp1", [R, 128], mybir.dt.bfloat16, kind="ExternalInput").ap()
    rp16 = nc.dram_tensor("rp16", [R, 128], mybir.dt.bfloat16, kind="ExternalInput").ap()
    out = nc.dram_tensor("out", [128, G * NSX * D], mybir.dt.int8, kind="ExternalOutput").ap()

    with tile.TileContext(nc) as tc:
        with contextlib.ExitStack() as ctx:
            cpool = ctx.enter_context(tc.tile_pool(name="consts", bufs=1))
            xpool = ctx.enter_context(tc.tile_pool(name="x", bufs=6))
            v8pool = ctx.enter_context(tc.tile_pool(name="xv8", bufs=2))
            vrpool = ctx.enter_context(tc.tile_pool(name="xvr", bufs=5))
            opool = ctx.enter_context(tc.tile_pool(name="ob", bufs=4))
            psA = ctx.enter_context(tc.tile_pool(name="psA", bufs=1, space="PSUM"))
            psR = ctx.enter_context(tc.tile_pool(name="psR", bufs=1, space="PSUM"))
            psB = ctx.enter_context(tc.tile_pool(name="psB", bufs=3, space="PSUM"))

            # consts head the FIFO ring; vt lands first for mm1(g0)
            vt_sb = cpool.tile([128, NCH * R], mybir.dt.bfloat16)
            nc.sync.dma_start(vt_sb[:], vt[:])
            wt_sb = cpool.tile([R, D], mybir.dt.bfloat16)
            nc.sync.dma_start(wt_sb[:], wt[:])
            rp1_sb = cpool.tile([R, 128], mybir.dt.bfloat16)
            nc.sync.dma_start(rp1_sb[:], rp1[:])
            rp16_sb = cpool.tile([R, 128], mybir.dt.bfloat16)
            nc.sync.dma_start(rp16_sb[:], rp16[:])
            wtr_sb = cpool.tile([128, D], mybir.dt.bfloat16)

            # the whole x stream, 512 KiB sub-DMAs, pushed ahead of all writes
            xs = []
            for t in range(G * TPG):
                xt = xpool.tile([128, CPT * SG], mybir.dt.bfloat16, tag="xs")
                for q in range(NSUB):
                    cols = slice(q * CPS * SG, (q + 1) * CPS * SG)
                    nc.sync.dma_start(xt[:, cols], xtp[t * 128:(t + 1) * 128, cols])
                xs.append(xt)

            # groups: (row_start, nrows); smaller tail groups shrink the
            # serial mm1->mm2->drain->write chain after the read stream ends
            GROUPS = [(0, 512), (512, 512), (1024, 512), (1536, 384), (1920, 128)]
            NG = len(GROUPS)
            obs = []
            for k, (rs, n) in enumerate(GROUPS):
                obs.append(opool.tile([128, (n // 128) * D], mybir.dt.int8,
                                      tag="ob", name=f"ob{k}"))
            ps8_cur = [None]
            xvr_of = {}

            def mm1_chunk(k, ch):
                rs, n = GROUPS[k]
                if ch == 0:
                    ps8_cur[0] = psA.tile([R, n], mybir.dt.float32, tag="ps8",
                                          name="ps8")
                t2, c = divmod(ch, CPT)
                sl = rs % SG
                xt = xs[(rs // SG) * TPG + t2]
                nc.tensor.matmul(
                    ps8_cur[0][:],
                    vt_sb[:, ch * R:(ch + 1) * R],
                    xt[:, c * SG + sl:c * SG + sl + n],
                    start=(ch == 0),
                    stop=(ch == NCH - 1),
                )

            def finish_a(k):
                # ps_xv8 -> SBUF (DVE), overlaps trailing mm2 units on PE
                rs, n = GROUPS[k]
                xv8 = v8pool.tile([R, n], mybir.dt.bfloat16, tag="xv8", name="xv8")
                nc.vector.tensor_copy(xv8[:], ps8_cur[0][:])
                return xv8

            def finish_b(k, xv8):
                # one-matmul 16x partition replication
                rs, n = GROUPS[k]
                ps2 = psR.tile([128, n], mybir.dt.float32, tag="ps2", name="ps2")
                nc.tensor.matmul(ps2[:], rp1_sb[:], xv8[:], start=True, stop=True)
                xvr = vrpool.tile([128, n], mybir.dt.bfloat16, tag="xvr", name="xvr")
                nc.scalar.copy(xvr[:], ps2[:])
                xvr_of[k] = xvr

            def wtr_unit(u):
                # wtr = wt[p%8]/16, built by repmat matmuls, paired drains
                psw = psB.tile([128, 1024], mybir.dt.float32, tag="psb", name="psw")
                for h in range(2):
                    j = u * 2 + h
                    nc.tensor.matmul(
                        psw[:, h * 512:(h + 1) * 512], rp16_sb[:],
                        wt_sb[:, j * 512:(j + 1) * 512], start=True, stop=True,
                    )
                dst = wtr_sb[:, u * 1024:(u + 1) * 1024]
                if u % 2 == 0:
                    nc.vector.tensor_copy(dst, psw[:])
                else:
                    nc.scalar.copy(dst, psw[:])

            def mm2_unit(k, u):
                # one paired-bank unit: [128, 1024] of out strip u//NDJ
                i, uu = divmod(u, NDJ)
                xvr = xvr_of[k]
                ps = psB.tile([128, 1024], mybir.dt.float32, tag="psb", name="ps")
                for h in range(2):
                    j = uu * 2 + h
                    nc.tensor.matmul(
                        ps[:, h * 512:(h + 1) * 512],
                        xvr[:, i * 128:(i + 1) * 128],
                        wtr_sb[:, j * 512:(j + 1) * 512],
                        start=True, stop=True,
                    )
                dst = obs[k][:, i * D + uu * 1024:i * D + (uu + 1) * 1024]
                if u % 2 == 0:
                    nc.vector.tensor_copy(dst, ps[:])
                else:
                    nc.scalar.copy(dst, ps[:])

            def out_off(k, i):
                rs, n = GROUPS[k]
                return (rs // 128 + i) * D

            # ---- build-time planner: emit PE work in data-arrival order,
            # filling predicted read-stall gaps with mm2 units ----
            MMNS = 216.0      # warm N=512 matmul cadence
            DRN = {0: 1223.0, 1: 1113.0}   # DVE / ACT [128,1024] drain
            T0, SUBNS = 11500.0, 524288 / 0.36   # first data; ns per 512KiB

            def sub_done(k, ch):
                rs, _ = GROUPS[k]
                t = (rs // SG) * TPG + ch // CPT
                q = (ch % CPT) // CPS
                return T0 + (t * NSUB + q + 1) * SUBNS

            eng_t = {0: 0.0, 1: 0.0}   # DVE, ACT sim time
            drain_ends = []            # last psB drain end times
            tog = [0]

            def sim_drain(t_pe, cols):
                e = tog[0]; tog[0] ^= 1
                tend = max(eng_t[e], t_pe) + DRN[e] * (cols / 1024.0)
                eng_t[e] = tend
                drain_ends.append(tend)
                return tend

            t_pe = 0.0
            unit_q = []                # (k, u) available for emission
            wtr_left = list(range(D // 1024))

            def emit_unit():
                nonlocal t_pe
                if wtr_left:
                    u = wtr_left.pop(0)
                    wtr_unit(u)
                    t_pe += 2 * MMNS
                    sim_drain(t_pe, 1024)
                    return True
                if unit_q and unit_q[0][0] <= t_pe:
                    _, k, u = unit_q.pop(0)
                    rs, n = GROUPS[k]
                    mm2_unit(k, u)
                    t_pe += 2 * MMNS
                    # psB backpressure: 3 bufs
                    if len(drain_ends) >= 3:
                        t_pe = max(t_pe, drain_ends[-3])
                    sim_drain(t_pe, 1024)
                    # group write on the sync ring (FIFO behind all reads)
                    if u == (n // 128) * NDJ - 1:
                        nc.sync.dma_start(
                            out[:, out_off(k, 0):out_off(k, 0) + (n // 128) * D],
                            obs[k][:])
                    return True
                return False

            for k, (rs, n) in enumerate(GROUPS):
                w = n / 512.0
                for ch in range(NCH):
                    ready = sub_done(k, ch)
                    # fill predicted PE idle with mm2/wtr units
                    while t_pe + 2 * MMNS < ready and emit_unit():
                        pass
                    mm1_chunk(k, ch)
                    t_pe = max(t_pe, ready) + MMNS * w
                xv8 = finish_a(k)
                # cover xv8-copy latency with one fill unit if possible
                emit_unit()
                finish_b(k, xv8)
                t_pe += MMNS * w + 600.0   # repl MM + chain slack
                avail = t_pe + 1500.0
                for u in range((n // 128) * NDJ):
                    unit_q.append((avail, k, u))
            while unit_q or wtr_left:
                if unit_q:
                    t_pe = max(t_pe, unit_q[0][0])
                if not emit_unit():
                    break

    nc.compile()
    return nc


def _get_kernels():
    if "k" not in _cache:
        _cache["k"] = build_fused()
    return _cache["k"]


def _vt_layout(V, d, r):
    """[128, (d//128)*r] bf16 with vt[p, c*r + j] = V[j, c*128 + p]."""
    nch = d // 128
    return np.ascontiguousarray(
        V.reshape(r, nch, 128).transpose(2, 1, 0).reshape(128, nch * r)
    ).astype(BF16)


def _routing_weights(x, V_shared, U_shared, core_pool, core_keys):
    """Exact f64 routing on host -> per-batch (W'_b [R, D] bf16 scaled by
    1/s_d, s [B, D] f32 dequant scales)."""
    mean = x.mean(axis=1, dtype=np.float64)  # [B, D]
    centroid = 0.7 * x[:, -1, :].astype(np.float64) + 0.3 * mean
    c_n = centroid / np.maximum(
        np.linalg.norm(centroid, axis=-1, keepdims=True), EPS
    )
    kk = core_keys.astype(np.float64)
    k_n = kk / np.maximum(np.linalg.norm(kk, axis=-1, keepdims=True), EPS)
    sim = c_n @ k_n.T  # [B, K]
    logits = sim / TEMP
    e = np.exp(logits - logits.max(axis=-1, keepdims=True))
    w = e / e.sum(axis=-1, keepdims=True)
    Lam = np.einsum("bk,kij->bij", w, core_pool.astype(np.float64))  # [B, R, R]
    Wb = np.einsum("dr,brj->bjd", U_shared.astype(np.float64), Lam)  # [B, R, D]
    Vf = V_shared.astype(np.float64)
    C = Vf @ Vf.T  # [R, R]
    sig = np.sqrt(np.einsum("bjd,jk,bkd->bd", Wb, C, Wb))  # [B, D]
    s = (QC / 127.0) * np.maximum(sig, 1e-12)  # [B, D]
    wt_b = [np.ascontiguousarray(Wb[b] / s[b][None, :]).astype(BF16) for b in range(B)]
    return wt_b, s.astype(np.float32)


def _pack_xtp(xshard):
    """[SH, D] f32 -> [1024, 8192] bf16: tile t=g*2+t2 row p col c*SG+s
    = x[g*512 + s, (t2*16 + c)*128 + p]."""
    v = np.ascontiguousarray(
        xshard.reshape(G, SG, TPG, CPT, 128).transpose(0, 2, 4, 3, 1)
    )
    return v.reshape(G * TPG * 128, CPT * SG).astype(BF16)


def _rp_layout(r, scale):
    """[r, 128] bf16, rp[k, m] = (m % r == k)*scale: partition replicator."""
    m = np.arange(128)
    return ((m[None, :] % r == np.arange(r)[:, None]) * scale).astype(BF16)


def _shard_inputs(x, V_shared, U_shared, core_pool, core_keys):
    vt_np = _vt_layout(V_shared.astype(np.float32), D, R)
    rp1_np = _rp_layout(R, 1.0)
    rp16_np = _rp_layout(R, 1.0 / 16.0)
    wt_b, s = _routing_weights(x, V_shared, U_shared, core_pool, core_keys)
    in_maps = []
    for c in range(NCORES):
        b, h = c // 2, c % 2
        xtp_c = _pack_xtp(x[b, h * SH:(h + 1) * SH, :])
        in_maps.append({"xtp": xtp_c, "vt": vt_np, "wt": wt_b[b],
                        "rp1": rp1_np, "rp16": rp16_np})
    return in_maps, s


def kernel(x, V_shared, U_shared, core_pool, core_keys):
    x = np.asarray(x)
    V_shared = np.asarray(V_shared)
    U_shared = np.asarray(U_shared)
    core_pool = np.asarray(core_pool)
    core_keys = np.asarray(core_keys)

    nc = _get_kernels()
    core_ids = list(range(NCORES))
    in_maps, s = _shard_inputs(x, V_shared, U_shared, core_pool, core_keys)
    res = run_bass_kernel_spmd(nc, in_maps, core_ids).results

    out = np.empty((B, S, D), dtype=np.float32)
    for c in core_ids:
        b, h = c // 2, c % 2
        a = res[c]["out"].reshape(128, G, NSX, D).transpose(1, 2, 0, 3)
        out[b, h * SH:(h + 1) * SH, :] = (
            a.reshape(SH, D).astype(np.float32) * s[b][None, :]
        )
    return out


# revision 14
# speedup vs baseline: 1.2013x; 1.2013x over previous
"""Trainium2 Bass kernel for nn_CASCADES_v8_ResonantCore (moe_routing):

Computation (per batch b):
    centroid = 0.7*x[b,-1,:] + 0.3*mean_s(x[b])
    w = softmax(cos_sim(centroid, core_keys)/TEMP)      # [K]
    Lam = sum_k w[k] * core_pool[k]                     # [R,R]
    out[b] = ((x[b] @ V^T) @ Lam^T) @ U^T               # [S,D]

Strategy (8 cores, data-parallel over (batch, seq-half)):
  - Host: exact f64 routing; W_b = (U @ Lam_b)^T folded to one [R, D]
    weight per batch.  Output is written int8 with a per-column scale
    s_d = 8*sigma_d/127 (sigma_d^2 = W_d^T (V V^T) W_d) folded into
    the weight; host dequantizes.  f32->int8 on DVE/ACT rounds-to-
    nearest and saturates (HW-probed), so |err| <= s_d/2 ~ 0.03 sigma.
  - Reads: 512 KiB sub-DMAs on the sync ring (consts first), so the
    first matmul starts ~11.5 us instead of ~18.6 us.  Writes: eager
    per-strip int8 on the scalar ring.
  - Device per seq-group: V replicated 16x along the free dim in SBUF,
    so 32 accumulating matmuls produce the replicated xv^T [128, n]
    directly in PSUM; one copy to SBUF; then 8 expansion matmuls per
    128-row strip with [128,512] f32->int8 drains alternating DVE/ACT.
  - Groups are (512,512,512,256,256) rows: the smaller tail groups
    shrink the serial mm1->mm2->drain->write chain after the 16.8 MiB
    read stream ends (~358 GB/s sustained per core).
  - HBM traffic per core: 16.8 MiB read + 8.4 MiB write ~= 25.3 MiB.
"""

import sys

sys.path.insert(0, "/opt/trn_rl_repo")

import contextlib

import ml_dtypes
import numpy as np

import concourse.bass as bass  # noqa: F401  (registers bass types)
import concourse.tile as tile
from concourse import bacc, mybir
from concourse.bass_utils import run_bass_kernel_spmd

BF16 = ml_dtypes.bfloat16

B, S, D, R, K = 4, 4096, 4096, 8, 4
NCORES = 8
SH = S // 2     # 2048 seq rows per core
G = 4           # 512-row read-tile groups per core
SG = SH // G    # 512
NCH = D // 128  # 32 d-chunks
TPG = 2         # x tiles per 512-row group ([128, 8192] each)
CPT = NCH // TPG  # 16 d-chunks per x tile
NSUB = 4        # read sub-DMAs per tile (512 KiB each)
CPS = CPT // NSUB
EPS, TEMP = 1e-8, 0.05
QC = 8.0        # int8 scale: s_d = QC * sigma_d / 127

_cache = {}


def build_fused():
    """xtp [1024, 8192] bf16, vt [128, 256] bf16, wt [8, 4096] bf16
    (per-column-scaled W'), rp [8, 128] bf16 -> out [128, 65536] int8
    with out[p, q*4096 + d] = out_rows[q*128 + p, d], q = row-block."""
    rep = 128 // R
    nc = bacc.Bacc("TRN2", target_bir_lowering=False, debug=False)
    xtp = nc.dram_tensor(
        "xtp", [G * TPG * 128, CPT * SG], mybir.dt.bfloat16, kind="ExternalInput"
    ).ap()
    vt = nc.dram_tensor("vt", [128, NCH * R], mybir.dt.bfloat16, kind="ExternalInput").ap()
    wt = nc.dram_tensor("wt", [R, D], mybir.dt.bfloat16, kind="ExternalInput").ap()
    rp = nc.dram_tensor("rp", [R, 128], mybir.dt.bfloat16, kind="ExternalInput").ap()
    out = nc.dram_tensor("out", [128, (SH // 128) * D], mybir.dt.int8,
                         kind="ExternalOutput").ap()

    with tile.TileContext(nc) as tc:
        with contextlib.ExitStack() as ctx:
            cpool = ctx.enter_context(tc.tile_pool(name="consts", bufs=1))
            xpool = ctx.enter_context(tc.tile_pool(name="x", bufs=6))
            vrpool = ctx.enter_context(tc.tile_pool(name="xvr", bufs=2))
            opool = ctx.enter_context(tc.tile_pool(name="ob", bufs=8))
            psA = ctx.enter_context(tc.tile_pool(name="psA", bufs=2, space="PSUM"))
            psB = ctx.enter_context(tc.tile_pool(name="psB", bufs=6, space="PSUM"))

            # consts head the ring
            vt_sb = cpool.tile([128, NCH * R], mybir.dt.bfloat16)
            nc.sync.dma_start(vt_sb[:], vt[:])
            wt_sb = cpool.tile([R, D], mybir.dt.bfloat16)
            nc.sync.dma_start(wt_sb[:], wt[:])
            rp_sb = cpool.tile([R, 128], mybir.dt.bfloat16)
            nc.sync.dma_start(rp_sb[:], rp[:])

            # the whole x stream as 512 KiB sub-DMAs
            xs = []
            for t in range(G * TPG):
                xt = xpool.tile([128, CPT * SG], mybir.dt.bfloat16, tag="xs")
                for q in range(NSUB):
                    cols = slice(q * CPS * SG, (q + 1) * CPS * SG)
                    nc.sync.dma_start(xt[:, cols], xtp[t * 128:(t + 1) * 128, cols])
                xs.append(xt)

            # wtr = wt[p%8]/16 via 8 repmat matmuls (also HAM warmup)
            wtr_sb = cpool.tile([128, D], mybir.dt.bfloat16)
            for j in range(D // 512):
                psw = psB.tile([128, 512], mybir.dt.float32, tag="ps")
                nc.tensor.matmul(psw[:], rp_sb[:], wt_sb[:, j * 512:(j + 1) * 512],
                                 start=True, stop=True)
                if j % 2 == 0:
                    nc.vector.tensor_copy(wtr_sb[:, j * 512:(j + 1) * 512], psw[:])
                else:
                    nc.scalar.copy(wtr_sb[:, j * 512:(j + 1) * 512], psw[:])
            # vtr: V replicated 16x along the free dim, 16 strided copies
            vtr_sb = cpool.tile([128, NCH * 128], mybir.dt.bfloat16)
            vtr_v = vtr_sb[:].rearrange("p (c t j) -> p c t j", t=rep, j=R)
            vt_v = vt_sb[:].rearrange("p (c j) -> p c j", j=R)
            for t in range(rep):
                if t % 2 == 0:
                    nc.vector.tensor_copy(vtr_v[:, :, t, :], vt_v)
                else:
                    nc.scalar.copy(vtr_v[:, :, t, :], vt_v)

            GROUPS = [(0, 512), (512, 512), (1024, 512), (1536, 256), (1792, 256)]
            for k, (rs, n) in enumerate(GROUPS):
                # ---- mm1: replicated xv^T [128, n] over 32 d-chunks ----
                ps_xv = psA.tile([128, n], mybir.dt.float32, tag="psxv", name="psxv")
                sl = rs % SG
                for ch in range(NCH):
                    t2, c = divmod(ch, CPT)
                    xt = xs[(rs // SG) * TPG + t2]
                    nc.tensor.matmul(
                        ps_xv[:],
                        vtr_sb[:, ch * 128:(ch + 1) * 128],
                        xt[:, c * SG + sl:c * SG + sl + n],
                        start=(ch == 0),
                        stop=(ch == NCH - 1),
                    )
                xvr = vrpool.tile([128, n], mybir.dt.bfloat16, tag="xvr", name="xvr")
                if k % 2 == 0:
                    nc.vector.tensor_copy(xvr[:], ps_xv[:])
                else:
                    nc.scalar.copy(xvr[:], ps_xv[:])

                # ---- mm2: out strips [128, 4096] int8 = xv @ W'^T ----
                for i in range(n // 128):
                    ob = opool.tile([128, D], mybir.dt.int8, tag="ob", name="ob")
                    for j in range(D // 512):
                        ps = psB.tile([128, 512], mybir.dt.float32, tag="ps", name="ps")
                        nc.tensor.matmul(
                            ps[:],
                            xvr[:, i * 128:(i + 1) * 128],
                            wtr_sb[:, j * 512:(j + 1) * 512],
                            start=True, stop=True,
                        )
                        dst = ob[:, j * 512:(j + 1) * 512]
                        if j % 2 == 0:
                            nc.vector.tensor_copy(dst, ps[:])
                        else:
                            nc.scalar.copy(dst, ps[:])
                    q = rs // 128 + i
                    nc.scalar.dma_start(out[:, q * D:(q + 1) * D], ob[:])

    nc.compile()
    return nc


def _get_kernels():
    if "k" not in _cache:
        _cache["k"] = build_fused()
    return _cache["k"]


def _vt_layout(V, d, r):
    """[128, (d//128)*r] bf16 with vt[p, c*r + j] = V[j, c*128 + p]."""
    nch = d // 128
    return np.ascontiguousarray(
        V.reshape(r, nch, 128).transpose(2, 1, 0).reshape(128, nch * r)
    ).astype(BF16)


def _routing_weights(x, V_shared, U_shared, core_pool, core_keys):
    """Exact f64 routing on host -> per-batch (W'_b [R, D] bf16 scaled by
    1/s_d, s [B, D] f32 dequant scales)."""
    mean = x.mean(axis=1, dtype=np.float64)  # [B, D]
    centroid = 0.7 * x[:, -1, :].astype(np.float64) + 0.3 * mean
    c_n = centroid / np.maximum(
        np.linalg.norm(centroid, axis=-1, keepdims=True), EPS
    )
    kk = core_keys.astype(np.float64)
    k_n = kk / np.maximum(np.linalg.norm(kk, axis=-1, keepdims=True), EPS)
    sim = c_n @ k_n.T  # [B, K]
    logits = sim / TEMP
    e = np.exp(logits - logits.max(axis=-1, keepdims=True))
    w = e / e.sum(axis=-1, keepdims=True)
    Lam = np.einsum("bk,kij->bij", w, core_pool.astype(np.float64))  # [B, R, R]
    Wb = np.einsum("dr,brj->bjd", U_shared.astype(np.float64), Lam)  # [B, R, D]
    Vf = V_shared.astype(np.float64)
    C = Vf @ Vf.T  # [R, R]
    sig = np.sqrt(np.einsum("bjd,jk,bkd->bd", Wb, C, Wb))  # [B, D]
    s = (QC / 127.0) * np.maximum(sig, 1e-12)  # [B, D]
    wt_b = [np.ascontiguousarray(Wb[b] / s[b][None, :]).astype(BF16) for b in range(B)]
    return wt_b, s.astype(np.float32)


def _pack_xtp(xshard):
    """[SH, D] f32 -> [1024, 8192] bf16: tile t=g*2+t2 row p col c*SG+s
    = x[g*512 + s, (t2*16 + c)*128 + p]."""
    v = np.ascontiguousarray(
        xshard.reshape(G, SG, TPG, CPT, 128).transpose(0, 2, 4, 3, 1)
    )
    return v.reshape(G * TPG * 128, CPT * SG).astype(BF16)


def _rp_layout(r):
    """[r, 128] bf16, rp[k, m] = (m % r == k)/16: partition replicator."""
    m = np.arange(128)
    return ((m[None, :] % r == np.arange(r)[:, None]) / 16.0).astype(BF16)


def _shard_inputs(x, V_shared, U_shared, core_pool, core_keys):
    vt_np = _vt_layout(V_shared.astype(np.float32), D, R)
    rp_np = _rp_layout(R)
    wt_b, s = _routing_weights(x, V_shared, U_shared, core_pool, core_keys)
    in_maps = []
    for c in range(NCORES):
        b, h = c // 2, c % 2
        xtp_c = _pack_xtp(x[b, h * SH:(h + 1) * SH, :])
        in_maps.append({"xtp": xtp_c, "vt": vt_np, "wt": wt_b[b], "rp": rp_np})
    return in_maps, s


def kernel(x, V_shared, U_shared, core_pool, core_keys):
    x = np.asarray(x)
    V_shared = np.asarray(V_shared)
    U_shared = np.asarray(U_shared)
    core_pool = np.asarray(core_pool)
    core_keys = np.asarray(core_keys)

    nc = _get_kernels()
    core_ids = list(range(NCORES))
    in_maps, s = _shard_inputs(x, V_shared, U_shared, core_pool, core_keys)
    res = run_bass_kernel_spmd(nc, in_maps, core_ids).results

    out = np.empty((B, S, D), dtype=np.float32)
    for c in core_ids:
        b, h = c // 2, c % 2
        a = res[c]["out"].reshape(128, SH // 128, D).transpose(1, 0, 2)
        out[b, h * SH:(h + 1) * SH, :] = (
            a.reshape(SH, D).astype(np.float32) * s[b][None, :]
        )
    return out
